# revision 21
# baseline (speedup 1.0000x reference)
"""Trainium2 Bass kernel for nn_CommonFeatureExtractor (v3, b-major mid).

Data-parallel over 8 NeuronCores: batch dim (4096) sharded into 8 x 512,
weights replicated.

Layer-1 GEMMs run in the transposed layout (h.T [dh, b], fed by x.T);
layer-2 GEMMs flip to "b-major": lhsT = h.T[:, b-tile] so fps comes out
as [b(128-part) x h(free)] per b-tile of 128 samples.  In b-major every
per-sample scalar (pair dot d, norm ss, softmax weights wq/fpw, fallback
mf) is a [P,1] per-partition column, so:
  - d and ss fall out of fused accum_out on ops that compute the pair
    products / squares anyway (no ones-matmuls, no row DMAs);
  - softmax chains are tiny [128,10] ops; reciprocal is [128,4];
  - no partition-broadcast matmuls;
  - masked aggregation = chain of fused affine_then_add ops
    (acc = msum_p * wq_p[P,1] + acc), one DVE op per pair.

v4: all DVE mid-ops run at [P,512] granularity (HW-measured: TT=425ns,
TS=349ns, STT+accum=690ns reliably, while [P,2048] ops are bimodal
1.2-5us under engine/DMA contention).  Pair sums fps_i+fps_j ride
SBUF->SBUF DMA with accum_op=add (SDMA CCE adders, frees GpSimd).  The
softmax division is deferred: common = (sum_p e_p*msum_p)*(0.5/den') +
mf*S with e_p = [d>0]*exp(sims_p) PAIR-LOCAL, so the weighted
aggregation chains (affine_then_add, scale=e_p[P,1]) run progressively
DURING phase A as each pair completes instead of post-softmax.
"""

import numpy as np

import concourse.bass as bass
import concourse.mybir as mybir
import concourse.tile as tile
from concourse import bacc
from concourse.bass_utils import run_bass_kernel_spmd

F32 = mybir.dt.float32
FP16 = mybir.dt.float16
ALU = mybir.AluOpType
AF = mybir.ActivationFunctionType

N_CORES = 8
B = 4096
BC = B // N_CORES  # 512 samples per core
H = 512
P = 128
NBT = BC // P  # 4 b-tiles per core
BH = NBT * H   # flat free size of one [all-samples, H] slab

AP_D, MA_D, MB_D, MC_D, PH_D = 2048, 167, 2048, 2048, 27
ENCS = [
    ("ap", AP_D, 16, 512),
    ("ma", MA_D, 2, 256),
    ("mb", MB_D, 16, 512),
    ("mc", MC_D, 16, 512),
    ("ph", PH_D, 1, 128),
]
XT_K = sum(e[2] for e in ENCS)  # 51 padded k-tiles of x
XT_OFF = np.cumsum([0] + [e[2] for e in ENCS])[:-1]

_I = [0, 0, 0, 0, 1, 1, 1, 2, 2, 3]
_J = [1, 2, 3, 4, 2, 3, 4, 3, 4, 4]
PAIR_IDX = {(_I[p], _J[p]): p for p in range(10)}
# compute order: small encoders first so pair work overlaps phase A
ORDER = ["ma", "ph", "ap", "mb", "mc"]
ENC_BY_NAME = {e[0]: (i, e) for i, e in enumerate(ENCS)}
# pair-completion order given ORDER (aggregation chains: late msums last)
ORDER_PAIRS = [6, 0, 3, 4, 8, 1, 2, 5, 7, 9]

MID = FP16


def build_bass():
    nc = bacc.Bacc("TRN2", target_bir_lowering=False, debug=False)

    xt = nc.dram_tensor("xt", [XT_K * P, BC], FP16, kind="ExternalInput")
    w1 = {}
    w2 = {}
    b1 = {}
    b2r = {}
    wgp = {}
    for name, _, K, dh in ENCS:
        w1[name] = nc.dram_tensor(f"w1_{name}", [K * P, dh], FP16, kind="ExternalInput")
        w2[name] = nc.dram_tensor(f"w2_{name}", [dh, H], FP16, kind="ExternalInput")
        b1[name] = nc.dram_tensor(f"b1_{name}", [P, dh // P], F32, kind="ExternalInput")
        b2r[name] = nc.dram_tensor(f"b2r_{name}", [1, H], FP16, kind="ExternalInput")
        wgp[name] = nc.dram_tensor(f"wgp_{name}", [dh, 5], FP16, kind="ExternalInput")
    z0 = nc.dram_tensor("z0", [5, 1], F32, kind="ExternalInput")
    id128 = nc.dram_tensor("id128", [P, P], FP16, kind="ExternalInput")
    enh_w = nc.dram_tensor("enh_w", [H, H], FP16, kind="ExternalInput")
    enh_b = nc.dram_tensor("enh_b", [P, 4], F32, kind="ExternalInput")
    fus_w = nc.dram_tensor("fus_w", [2 * H, H], FP16, kind="ExternalInput")
    fus_b = nc.dram_tensor("fus_b", [P, 4], F32, kind="ExternalInput")
    out = nc.dram_tensor("out", [H, BC], F32, kind="ExternalOutput")

    with tile.TileContext(nc) as tc:
        kernel_body(
            tc, xt, w1, w2, b1, b2r, wgp, z0, id128, enh_w, enh_b, fus_w, fus_b, out
        )
    nc.compile()
    return nc


def kernel_body(tc, xt, w1, w2, b1, b2r, wgp, z0, id128, enh_w, enh_b, fus_w,
                fus_b, out):
    nc = tc.nc
    import contextlib

    ctx = contextlib.ExitStack()
    with ctx:
        # -------- pools --------
        persist = ctx.enter_context(tc.tile_pool(name="persist", bufs=1))
        scr_pool = ctx.enter_context(tc.tile_pool(name="scr", bufs=3))
        msk_pool = ctx.enter_context(tc.tile_pool(name="msk", bufs=2))
        cb_pool = ctx.enter_context(tc.tile_pool(name="cb", bufs=2))
        wt_pool = ctx.enter_context(tc.tile_pool(name="wt", bufs=3))
        xt_pool = ctx.enter_context(tc.tile_pool(name="xtp", bufs=3))
        w_pool = ctx.enter_context(tc.tile_pool(name="wp", bufs=3))
        h_pool = ctx.enter_context(tc.tile_pool(name="hp", bufs=2))
        sq_pool = ctx.enter_context(tc.tile_pool(name="sqp", bufs=2))
        gate_pool = ctx.enter_context(tc.tile_pool(name="gatep", bufs=2))
        psum_l1 = ctx.enter_context(tc.tile_pool(name="psl1", bufs=4, space="PSUM"))
        psum_l2 = ctx.enter_context(tc.tile_pool(name="psl2", bufs=3, space="PSUM"))
        psum_z = ctx.enter_context(tc.tile_pool(name="psz", bufs=1, space="PSUM"))

        # -------- persistent tiles --------
        fps_bt = persist.tile([P, 5, BH], MID)       # b-major fps, flat slabs
        msum = persist.tile([P, 10, BH], MID)        # masked pair sums
        s_b = persist.tile([P, BH], MID)             # sum_i fps_i
        stats = persist.tile([P, NBT, 16], F32)      # cols 0-9 d, 10-14 ss
        pl_t = persist.tile([P, NBT, 10], MID)
        lss_t = persist.tile([P, NBT, 5], MID)
        invnn_t = persist.tile([P, NBT, 10], MID)
        sims_t = persist.tile([P, NBT, 10], MID)
        es_t = persist.tile([P, NBT, 10], MID)
        e_t = persist.tile([P, NBT, 10], F32)
        den_t = persist.tile([P, NBT], F32)
        den1_t = persist.tile([P, NBT], F32)
        rden_t = persist.tile([P, NBT], F32)
        r05_t = persist.tile([P, NBT], F32)
        mf_t = persist.tile([P, NBT], F32)
        ez_bt = persist.tile([P, NBT, 5], F32)
        sez_t = persist.tile([P, NBT], F32)
        rsez_t = persist.tile([P, NBT], F32)
        fpw_t = persist.tile([P, NBT, 5], F32)
        common_b = persist.tile([P, BH], MID)
        numer_b = persist.tile([P, BH], MID)
        wsum_b = persist.tile([P, BH], MID)
        common_h = persist.tile([P, 4, BC], MID)
        wsum_h = persist.tile([P, 4, BC], MID)
        enh_sb = persist.tile([P, 4, BC], MID)
        id_sb = persist.tile([P, P], FP16)
        ones_row = persist.tile([1, P], FP16)
        warmz = persist.tile([1, BC], MID)
        b1_sb = {}
        b2r_sb = {}
        wgp_sb = {}
        for name, _, K, dh in ENCS:
            b1_sb[name] = persist.tile([P, dh // P], F32, name=f"b1sb_{name}")
            b2r_sb[name] = persist.tile([1, H], FP16, name=f"b2r_{name}")
            wgp_sb[name] = persist.tile([P, dh // P, 5], FP16, name=f"wgp_{name}")
        z0_sb = persist.tile([5, 1], F32)
        enhb_sb = persist.tile([P, 4], F32)
        fusb_sb = persist.tile([P, 4], F32)
        ew_t = persist.tile([P, 4, 512], FP16, name="ew_t")
        fw_lo = persist.tile([P, 4, 512], FP16, name="fw_lo")
        fw_hi = persist.tile([P, 4, 512], FP16, name="fw_hi")

        nc.vector.memset(ones_row, 1.0)
        nc.vector.memset(warmz, 0.0)
        # PE warmup during the DMA preamble (K=1 matmuls, no DMA deps)
        for _wu in range(16):
            wu_ps = psum_l2.tile([P, H], F32, tag="l2ps", name=f"warm{_wu}")
            nc.tensor.matmul(wu_ps, ones_row, warmz, start=True, stop=True)
        nc.gpsimd.dma_start(id_sb, id128.ap())
        for name, _, K, dh in ENCS:
            nc.scalar.dma_start(b1_sb[name], b1[name].ap())
            nc.scalar.dma_start(b2r_sb[name], b2r[name].ap())
            nc.gpsimd.dma_start(
                wgp_sb[name], wgp[name].ap().rearrange("(ko p) m -> p ko m", p=P)
            )
        nc.gpsimd.dma_start(z0_sb, z0.ap())
        nc.scalar.dma_start(enhb_sb, enh_b.ap())
        nc.scalar.dma_start(fusb_sb, fus_b.ap())

        xt_view = xt.ap().rearrange("(ko p) n -> p ko n", p=P)

        def fps_blk(i, bt):  # [P, H] flat slice of encoder i, b-tile bt
            return fps_bt[:, i, bt * H : (bt + 1) * H]

        # ================= Phase A: encoders + pair prep ==================
        z_ps = psum_z.tile([5, BC], F32, tag="zps", name="zgate")
        Z_MM_TOTAL = sum(e[3] // P for e in ENCS)  # 15
        z_mm_done = 0

        numer_chains = [[] for _ in range(NBT)]
        n_pairs_done = 0
        pending_group = None
        n_enc_done = 0
        done_encs = []
        for name in ORDER:
            ei, (_, _, K, dh) = ENC_BY_NAME[name]
            M = dh // P
            # ---- layer 1 (h-major) ----
            psums = [
                psum_l1.tile([P, BC], F32, tag="mmps", name=f"l1_{name}_{m}")
                for m in range(M)
            ]
            h_sb = h_pool.tile([P, 4, BC], FP16, tag="htile")
            kdone = 0
            for kc0 in range(0, K, 4):
                kn = min(4, K - kc0)
                xt_t = xt_pool.tile([P, 4, BC], FP16, tag="xt")
                nc.sync.dma_start(
                    xt_t[:, :kn, :],
                    xt_view[:, XT_OFF[ei] + kc0 : XT_OFF[ei] + kc0 + kn, :],
                )
                w1_t = w_pool.tile([P, 4, 512], FP16, tag="w1")
                nc.sync.dma_start(
                    w1_t[:, :kn, :dh],
                    w1[name].ap()[kc0 * P : (kc0 + kn) * P, :].rearrange(
                        "(ko p) m -> p ko m", p=P
                    ),
                )
                for m in range(M):
                    for k in range(kn):
                        nc.tensor.matmul(
                            psums[m],
                            w1_t[:, k, m * P : (m + 1) * P],
                            xt_t[:, k, :],
                            start=(kdone + k == 0),
                            stop=(kdone + k == K - 1),
                        )
                kdone += kn
            for m in range(M):
                nc.scalar.activation(
                    h_sb[:, m, :], psums[m], AF.Relu, bias=b1_sb[name][:, m : m + 1]
                )
            # ---- gate partial ----
            for k in range(M):
                nc.tensor.matmul(
                    z_ps,
                    wgp_sb[name][:, k, :],
                    h_sb[:, k, :],
                    start=(z_mm_done == 0),
                    stop=(z_mm_done + 1 == Z_MM_TOTAL),
                )
                z_mm_done += 1
            # ---- layer 2 (b-major) ----
            w2_t = w_pool.tile([P, 4, 512], FP16, tag="w1")
            nc.sync.dma_start(
                w2_t[:, :M, :], w2[name].ap().rearrange("(ko p) m -> p ko m", p=P)
            )
            for bt in range(NBT):
                ps = psum_l2.tile([P, H], F32, tag="l2ps", name=f"l2_{name}_{bt}")
                for k in range(M):
                    nc.tensor.matmul(
                        ps,
                        h_sb[:, k, bt * P : (bt + 1) * P],
                        w2_t[:, k, :],
                        start=(k == 0),
                        stop=False,
                    )
                nc.tensor.matmul(
                    ps, ones_row[0:1, :], b2r_sb[name][0:1, :], start=False, stop=True
                )
                nc.scalar.activation(fps_blk(ei, bt), ps, AF.Copy)
                sq = sq_pool.tile([P, H], MID, tag="sq")
                nc.scalar.activation(
                    sq,
                    fps_blk(ei, bt),
                    AF.Square,
                    accum_out=stats[:, bt, 10 + ei : 11 + ei],
                )
            # ---- per-encoder ln(ss) column for the pair-sim chain ----
            nc.scalar.activation(
                lss_t[:, :, ei : ei + 1], stats[:, :, 10 + ei : 11 + ei], AF.Ln
            )
            # ---- pair pipeline: prods (+d via accum) NOW; the mask/msum/
            # e-chain/numer work for the PREVIOUS encoder's pairs is emitted
            # one encoder late (grouped q-contiguous ops; keeps ACT table
            # swaps and DVE backlog off the PE-critical relu/evac path)
            q0 = n_pairs_done
            newpairs = []
            for prev in done_encs:
                pkey = (min(prev, ei), max(prev, ei))
                i1, i2 = pkey
                q = n_pairs_done
                prod = scr_pool.tile([P, BH], MID, tag="prod", name=f"prod{q}")
                for bt in range(NBT):
                    nc.vector.scalar_tensor_tensor(
                        prod[:, bt * H : (bt + 1) * H],
                        in0=fps_blk(i1, bt),
                        scalar=0.0,
                        in1=fps_blk(i2, bt),
                        op0=ALU.add,
                        op1=ALU.mult,
                        accum_out=stats[:, bt, q : q + 1],
                    )
                sum_t = scr_pool.tile([P, BH], MID, tag="sum", name=f"sum{q}")
                nc.gpsimd.tensor_add(sum_t, fps_bt[:, i1, :], fps_bt[:, i2, :])
                # msum now (frees prod/sum scratch within the pair idiom);
                # 3 earliest pairs' muls ride gpsimd (wide)
                if q < 3:
                    nc.vector.tensor_scalar(
                        prod, in0=prod, scalar1=0.0, scalar2=None, op0=ALU.is_gt
                    )
                    nc.gpsimd.tensor_mul(msum[:, q, :], prod, sum_t)
                else:
                    for bt in range(NBT):
                        pslc = prod[:, bt * H : (bt + 1) * H]
                        nc.vector.tensor_scalar(
                            pslc, in0=pslc,
                            scalar1=0.0, scalar2=None, op0=ALU.is_gt,
                        )
                        nc.vector.tensor_mul(
                            msum[:, q, bt * H : (bt + 1) * H], pslc,
                            sum_t[:, bt * H : (bt + 1) * H],
                        )
                newpairs.append((q, i1, i2))
                n_pairs_done += 1

            def emit_pair_group(group, g0, g1):
                # e_q = [d>0]*exp(d/(||a||*||b||)), grouped over g0:g1
                for q, i1, i2 in group:
                    nc.vector.tensor_add(
                        pl_t[:, :, q : q + 1],
                        lss_t[:, :, i1 : i1 + 1],
                        lss_t[:, :, i2 : i2 + 1],
                    )
                nc.scalar.activation(
                    invnn_t[:, :, g0:g1], pl_t[:, :, g0:g1], AF.Exp, scale=-0.5
                )
                nc.vector.tensor_mul(
                    sims_t[:, :, g0:g1], stats[:, :, g0:g1], invnn_t[:, :, g0:g1]
                )
                nc.scalar.activation(es_t[:, :, g0:g1], sims_t[:, :, g0:g1], AF.Exp)
                nc.vector.scalar_tensor_tensor(
                    e_t[:, :, g0:g1],
                    in0=stats[:, :, g0:g1],
                    scalar=0.0,
                    in1=es_t[:, :, g0:g1],
                    op0=ALU.is_gt,
                    op1=ALU.mult,
                )
                for q, i1, i2 in group:
                    for bt in range(NBT):
                        mslc = msum[:, q, bt * H : (bt + 1) * H]
                        nch = numer_chains[bt]
                        if len(nch) == 0:
                            seed = cb_pool.tile([P, H], MID, tag=f"nb{bt}",
                                                name=f"nseed{bt}")
                            nc.vector.tensor_scalar(
                                seed, in0=mslc,
                                scalar1=e_t[:, bt, q : q + 1], scalar2=None,
                                op0=ALU.mult,
                            )
                            nch.append(seed)
                        else:
                            lastn = len(nch)
                            dst = (
                                numer_b[:, bt * H : (bt + 1) * H]
                                if lastn == 9
                                else cb_pool.tile([P, H], MID, tag=f"nb{bt}",
                                                  name=f"nb{bt}_{lastn}")
                            )
                            nc.vector.affine_then_add(
                                dst, mslc, nch[-1],
                                scale=e_t[:, bt, q : q + 1], bias=0.0,
                            )
                            nch.append(dst)

            if pending_group is not None:
                emit_pair_group(*pending_group)
            pending_group = (newpairs, q0, n_pairs_done) if newpairs else None
            if name == ORDER[-1] and pending_group is not None:
                emit_pair_group(*pending_group)
                pending_group = None
            # ---- running S on gpsimd ----
            cur = fps_bt[:, ei, :]
            if n_enc_done == 0:
                s_prev = cur
            elif n_enc_done == len(ORDER) - 1:
                nc.gpsimd.tensor_add(s_b, s_prev, cur)
            else:
                s_new = msk_pool.tile([P, BH], MID, tag="sacc",
                                      name=f"sacc{n_enc_done}")
                nc.gpsimd.tensor_add(s_new, s_prev, cur)
                s_prev = s_new
            done_encs.append(ei)
            n_enc_done += 1
            if n_enc_done == 2:
                nc.sync.dma_start(
                    ew_t, enh_w.ap().rearrange("(ko p) m -> p ko m", p=P)
                )
                fw_view = fus_w.ap().rearrange("(ko p) m -> p ko m", p=P)
                nc.sync.dma_start(fw_lo, fw_view[:, 0:4, :])
                nc.sync.dma_start(fw_hi, fw_view[:, 4:8, :])

        # ================= Phase B =================
        # fpw chain (needs z only)
        ez_h = cb_pool.tile([5, BC], MID, tag="ezh", name="ez_h")
        nc.scalar.activation(ez_h, z_ps, AF.Exp, bias=z0_sb[0:5, :])
        for bt in range(NBT):
            tps = psum_l1.tile([P, 8], FP16, tag="mmps", name=f"ezT{bt}")
            nc.tensor.transpose(
                tps[:, 0:5], ez_h[0:5, bt * P : (bt + 1) * P], id_sb[0:5, 0:5]
            )
            nc.scalar.activation(
                ez_bt[:, bt, :], tps[:, 0:5], AF.Copy,
                accum_out=sez_t[:, bt : bt + 1],
            )
        nc.vector.reciprocal(rsez_t, sez_t)

        # wsum chains: muls on ACT with UNNORMALIZED ez scale, adds on DVE,
        # then one TS by 1/sez
        for bt in range(NBT):
            wts = []
            for i in range(5):
                wt = wt_pool.tile([P, H], MID, tag="wt", name=f"wt{bt}_{i}")
                nc.scalar.activation(
                    wt, fps_blk(i, bt), AF.Copy, scale=ez_bt[:, bt, i : i + 1]
                )
                wts.append(wt)
            u1 = cb_pool.tile([P, H], MID, tag="wu", name=f"wu1_{bt}")
            u2 = cb_pool.tile([P, H], MID, tag="wu2", name=f"wu2_{bt}")
            nc.vector.tensor_add(u1, wts[0], wts[1])
            nc.vector.tensor_add(u2, wts[2], wts[3])
            u3 = cb_pool.tile([P, H], MID, tag="wu", name=f"wu3_{bt}")
            nc.vector.tensor_add(u3, u1, u2)
            u4 = cb_pool.tile([P, H], MID, tag="wu2", name=f"wu4_{bt}")
            nc.vector.tensor_add(u4, u3, wts[4])
            nc.vector.tensor_scalar(
                wsum_b[:, bt * H : (bt + 1) * H], in0=u4,
                scalar1=rsez_t[:, bt : bt + 1], scalar2=None, op0=ALU.mult,
            )
            # transpose wsum b-tile to h-major right away
            for ht in range(4):
                tps = psum_l1.tile([P, P], FP16, tag="mmps", name=f"wT{bt}_{ht}")
                nc.tensor.transpose(
                    tps, wsum_b[:, bt * H + ht * P : bt * H + (ht + 1) * P], id_sb
                )
                nc.scalar.activation(
                    wsum_h[:, ht, bt * P : (bt + 1) * P], tps, AF.Copy
                )

        # den / mf / common finishing (per-pair e columns already done)
        for bt in range(NBT):
            dj = sq_pool.tile([P, 16], MID, tag="dj", name=f"den{bt}")
            nc.vector.tensor_scalar(
                dj[:, 0:10], in0=e_t[:, bt, :], scalar1=0.0, scalar2=0.0,
                op0=ALU.add, op1=ALU.add, accum_out=den_t[:, bt : bt + 1],
            )
        nc.vector.tensor_scalar(
            mf_t, in0=den_t, scalar1=0.0, scalar2=0.2, op0=ALU.is_le, op1=ALU.mult
        )
        nc.vector.tensor_scalar_max(den1_t, den_t, 1.0)
        nc.vector.reciprocal(rden_t, den1_t)
        nc.vector.tensor_scalar_mul(r05_t, rden_t, 0.5)

        # ================= Phase C: finish common per b-tile ==============
        for bt in range(NBT):
            ctmp = cb_pool.tile([P, H], MID, tag="cb", name=f"ctmp{bt}")
            nc.vector.tensor_scalar(
                ctmp, in0=numer_b[:, bt * H : (bt + 1) * H],
                scalar1=r05_t[:, bt : bt + 1], scalar2=None, op0=ALU.mult,
            )
            nc.vector.affine_then_add(
                common_b[:, bt * H : (bt + 1) * H],
                s_b[:, bt * H : (bt + 1) * H], ctmp,
                scale=mf_t[:, bt : bt + 1], bias=0.0,
            )
            for ht in range(4):
                tps = psum_l1.tile([P, P], FP16, tag="mmps", name=f"cT{bt}_{ht}")
                nc.tensor.transpose(
                    tps, common_b[:, bt * H + ht * P : bt * H + (ht + 1) * P], id_sb
                )
                nc.scalar.activation(
                    common_h[:, ht, bt * P : (bt + 1) * P], tps, AF.Copy
                )

        # fus-lo accumulation (PE work spread through C); psums held open
        fus_ps = [
            psum_l2.tile([P, BC], F32, tag="l2ps", name=f"fus_{m}")
            for m in range(3)
        ]
        fus_ps.append(psum_z.tile([P, BC], F32, tag="zps", name="fus_3"))
        for m in range(4):
            for k in range(4):
                nc.tensor.matmul(
                    fus_ps[m],
                    fw_lo[:, k, m * P : (m + 1) * P],
                    wsum_h[:, k, :],
                    start=(k == 0),
                    stop=False,
                )

        # ================= Phase D: enhance + fuse (h-major) ===============
        for m in range(4):
            ps = psum_l1.tile([P, BC], F32, tag="mmps", name=f"enh_{m}")
            for k in range(4):
                nc.tensor.matmul(
                    ps,
                    ew_t[:, k, m * P : (m + 1) * P],
                    common_h[:, k, :],
                    start=(k == 0),
                    stop=(k == 3),
                )
            gate = gate_pool.tile([P, BC], MID, tag="gate")
            nc.scalar.activation(gate, ps, AF.Sigmoid, bias=enhb_sb[:, m : m + 1])
            nc.vector.tensor_mul(enh_sb[:, m, :], common_h[:, m, :], gate)

        out_view = out.ap().rearrange("(m p) n -> p m n", p=P)
        for m in range(4):
            for k in range(4):
                nc.tensor.matmul(
                    fus_ps[m],
                    fw_hi[:, k, m * P : (m + 1) * P],
                    enh_sb[:, k, :],
                    start=False,
                    stop=(k == 3),
                )
            o_sb = gate_pool.tile([P, BC], F32, tag="osb")
            nc.scalar.activation(
                o_sb, fus_ps[m], AF.Identity, bias=fusb_sb[:, m : m + 1]
            )
            nc.sync.dma_start(out_view[:, m, :], o_sb)


def prep_inputs(inputs):
    """Host-side: build the per-core in_maps from full inputs."""
    f16 = np.float16
    x = np.asarray(inputs["fp_features"], np.float32)

    def pad_rows(a, rows):
        a = np.asarray(a, np.float32)
        if a.shape[0] == rows:
            return a
        outp = np.zeros((rows, a.shape[1]), np.float32)
        outp[: a.shape[0]] = a
        return outp

    xt_full = np.zeros((XT_K * P, B), np.float32)
    offs_in = np.cumsum([0, AP_D, MA_D, MB_D, MC_D])
    for ei, (name, din, K, dh) in enumerate(ENCS):
        seg = x[:, offs_in[ei] : offs_in[ei] + din]
        xt_full[XT_OFF[ei] * P : XT_OFF[ei] * P + din, :] = np.ascontiguousarray(seg.T)
    xt_full = xt_full.astype(f16)

    wg_w = np.asarray(inputs["wg_w"], np.float32)
    wg_b = np.asarray(inputs["wg_b"], np.float32)
    common_map = {}
    z0v = wg_b.reshape(5).copy()
    for ei, (name, din, K, dh) in enumerate(ENCS):
        common_map[f"w1_{name}"] = pad_rows(inputs[f"{name}_w1"], K * P).astype(f16)
        w2f = np.asarray(inputs[f"{name}_w2"], np.float32)
        common_map[f"w2_{name}"] = w2f.astype(f16)
        common_map[f"b1_{name}"] = (
            np.asarray(inputs[f"{name}_b1"], np.float32).reshape(dh // P, P).T.copy()
        )
        b2f = np.asarray(inputs[f"{name}_b2"], np.float32)
        common_map[f"b2r_{name}"] = b2f.reshape(1, H).astype(f16)
        wg_i = wg_w[ei * H : (ei + 1) * H, :]
        common_map[f"wgp_{name}"] = (w2f @ wg_i).astype(f16)
        z0v += b2f @ wg_i
    common_map["z0"] = z0v.reshape(5, 1).astype(np.float32)
    common_map["id128"] = np.eye(P, dtype=f16)
    common_map["enh_w"] = np.asarray(inputs["enh_w"], f16)
    common_map["enh_b"] = np.asarray(inputs["enh_b"], np.float32).reshape(4, P).T.copy()
    common_map["fus_w"] = np.asarray(inputs["fus_w"], f16)
    common_map["fus_b"] = np.asarray(inputs["fus_b"], np.float32).reshape(4, P).T.copy()

    in_maps = []
    for c in range(N_CORES):
        m = dict(common_map)
        m["xt"] = np.ascontiguousarray(xt_full[:, c * BC : (c + 1) * BC])
        in_maps.append(m)
    return in_maps


_NC_CACHE = None


def kernel(**inputs) -> np.ndarray:
    global _NC_CACHE
    if _NC_CACHE is None:
        _NC_CACHE = build_bass()
    nc = _NC_CACHE
    in_maps = prep_inputs(inputs)
    res = run_bass_kernel_spmd(nc, in_maps, core_ids=list(range(N_CORES)))
    outs = [res.results[c]["out"] for c in range(N_CORES)]
    full = np.concatenate([o.T for o in outs], axis=0)
    return np.ascontiguousarray(full.astype(np.float32))


# revision 22
# speedup vs baseline: 1.0747x; 1.0747x over previous
"""Trainium2 Bass kernel for nn_CommonFeatureExtractor (v3, b-major mid).

Data-parallel over 8 NeuronCores: batch dim (4096) sharded into 8 x 512,
weights replicated.

Layer-1 GEMMs run in the transposed layout (h.T [dh, b], fed by x.T);
layer-2 GEMMs flip to "b-major": lhsT = h.T[:, b-tile] so fps comes out
as [b(128-part) x h(free)] per b-tile of 128 samples.  In b-major every
per-sample scalar (pair dot d, norm ss, softmax weights wq/fpw, fallback
mf) is a [P,1] per-partition column, so:
  - d and ss fall out of fused accum_out on ops that compute the pair
    products / squares anyway (no ones-matmuls, no row DMAs);
  - softmax chains are tiny [128,10] ops; reciprocal is [128,4];
  - no partition-broadcast matmuls;
  - masked aggregation = chain of fused affine_then_add ops
    (acc = msum_p * wq_p[P,1] + acc), one DVE op per pair.

v4: all DVE mid-ops run at [P,512] granularity (HW-measured: TT=425ns,
TS=349ns, STT+accum=690ns reliably, while [P,2048] ops are bimodal
1.2-5us under engine/DMA contention).  Pair sums fps_i+fps_j ride
SBUF->SBUF DMA with accum_op=add (SDMA CCE adders, frees GpSimd).  The
softmax division is deferred: common = (sum_p e_p*msum_p)*(0.5/den') +
mf*S with e_p = [d>0]*exp(sims_p) PAIR-LOCAL, so the weighted
aggregation chains (affine_then_add, scale=e_p[P,1]) run progressively
DURING phase A as each pair completes instead of post-softmax.
"""

import numpy as np

import concourse.bass as bass
import concourse.mybir as mybir
import concourse.tile as tile
from concourse import bacc
from concourse.bass_utils import run_bass_kernel_spmd

F32 = mybir.dt.float32
FP16 = mybir.dt.float16
ALU = mybir.AluOpType
AF = mybir.ActivationFunctionType

N_CORES = 8
B = 4096
BC = B // N_CORES  # 512 samples per core
H = 512
P = 128
NBT = BC // P  # 4 b-tiles per core
BH = NBT * H   # flat free size of one [all-samples, H] slab

AP_D, MA_D, MB_D, MC_D, PH_D = 2048, 167, 2048, 2048, 27
ENCS = [
    ("ap", AP_D, 16, 512),
    ("ma", MA_D, 2, 256),
    ("mb", MB_D, 16, 512),
    ("mc", MC_D, 16, 512),
    ("ph", PH_D, 1, 128),
]
XT_K = sum(e[2] for e in ENCS)  # 51 padded k-tiles of x
XT_OFF = np.cumsum([0] + [e[2] for e in ENCS])[:-1]

_I = [0, 0, 0, 0, 1, 1, 1, 2, 2, 3]
_J = [1, 2, 3, 4, 2, 3, 4, 3, 4, 4]
PAIR_IDX = {(_I[p], _J[p]): p for p in range(10)}
# compute order: small encoders first so pair work overlaps phase A
ORDER = ["ma", "ph", "ap", "mb", "mc"]
ENC_BY_NAME = {e[0]: (i, e) for i, e in enumerate(ENCS)}
# pair-completion order given ORDER (aggregation chains: late msums last)
ORDER_PAIRS = [6, 0, 3, 4, 8, 1, 2, 5, 7, 9]

MID = FP16


def build_bass():
    nc = bacc.Bacc("TRN2", target_bir_lowering=False, debug=False)

    xt = nc.dram_tensor("xt", [XT_K * P, BC], FP16, kind="ExternalInput")
    w1 = {}
    w2 = {}
    b1 = {}
    b2r = {}
    wgp = {}
    for name, _, K, dh in ENCS:
        w1[name] = nc.dram_tensor(f"w1_{name}", [K * P, dh], FP16, kind="ExternalInput")
        w2[name] = nc.dram_tensor(f"w2_{name}", [dh, H], FP16, kind="ExternalInput")
        b1[name] = nc.dram_tensor(f"b1_{name}", [P, dh // P], F32, kind="ExternalInput")
        b2r[name] = nc.dram_tensor(f"b2r_{name}", [1, H], FP16, kind="ExternalInput")
        wgp[name] = nc.dram_tensor(f"wgp_{name}", [dh, 5], FP16, kind="ExternalInput")
    z0 = nc.dram_tensor("z0", [5, 1], F32, kind="ExternalInput")
    id128 = nc.dram_tensor("id128", [P, P], FP16, kind="ExternalInput")
    enh_w = nc.dram_tensor("enh_w", [H, H], FP16, kind="ExternalInput")
    enh_b = nc.dram_tensor("enh_b", [P, 4], F32, kind="ExternalInput")
    fus_w = nc.dram_tensor("fus_w", [2 * H, H], FP16, kind="ExternalInput")
    fus_b = nc.dram_tensor("fus_b", [P, 4], F32, kind="ExternalInput")
    out = nc.dram_tensor("out", [H, BC], F32, kind="ExternalOutput")

    with tile.TileContext(nc) as tc:
        kernel_body(
            tc, xt, w1, w2, b1, b2r, wgp, z0, id128, enh_w, enh_b, fus_w, fus_b, out
        )
    nc.compile()
    return nc


def kernel_body(tc, xt, w1, w2, b1, b2r, wgp, z0, id128, enh_w, enh_b, fus_w,
                fus_b, out):
    nc = tc.nc
    import contextlib

    ctx = contextlib.ExitStack()
    with ctx:
        # -------- pools --------
        persist = ctx.enter_context(tc.tile_pool(name="persist", bufs=1))
        scr_pool = ctx.enter_context(tc.tile_pool(name="scr", bufs=3))
        msk_pool = ctx.enter_context(tc.tile_pool(name="msk", bufs=2))
        cb_pool = ctx.enter_context(tc.tile_pool(name="cb", bufs=2))
        wt_pool = ctx.enter_context(tc.tile_pool(name="wt", bufs=3))
        xt_pool = ctx.enter_context(tc.tile_pool(name="xtp", bufs=3))
        w_pool = ctx.enter_context(tc.tile_pool(name="wp", bufs=3))
        h_pool = ctx.enter_context(tc.tile_pool(name="hp", bufs=2))
        sq_pool = ctx.enter_context(tc.tile_pool(name="sqp", bufs=2))
        gate_pool = ctx.enter_context(tc.tile_pool(name="gatep", bufs=2))
        psum_l1 = ctx.enter_context(tc.tile_pool(name="psl1", bufs=4, space="PSUM"))
        psum_l2 = ctx.enter_context(tc.tile_pool(name="psl2", bufs=3, space="PSUM"))
        psum_z = ctx.enter_context(tc.tile_pool(name="psz", bufs=1, space="PSUM"))

        # -------- persistent tiles --------
        fps_bt = persist.tile([P, 5, BH], MID)       # b-major fps, flat slabs
        msum = persist.tile([P, 10, BH], MID)        # masked pair sums
        s_b = persist.tile([P, BH], MID)             # sum_i fps_i
        stats = persist.tile([P, NBT, 16], F32)      # cols 0-9 d, 10-14 ss
        pl_t = persist.tile([P, NBT, 10], MID)
        lss_t = persist.tile([P, NBT, 5], MID)
        invnn_t = persist.tile([P, NBT, 10], MID)
        sims_t = persist.tile([P, NBT, 10], MID)
        es_t = persist.tile([P, NBT, 10], MID)
        e_t = persist.tile([P, NBT, 10], F32)
        den_t = persist.tile([P, NBT], F32)
        den1_t = persist.tile([P, NBT], F32)
        rden_t = persist.tile([P, NBT], F32)
        r05_t = persist.tile([P, NBT], F32)
        mf_t = persist.tile([P, NBT], F32)
        ez_bt = persist.tile([P, NBT, 5], F32)
        sez_t = persist.tile([P, NBT], F32)
        rsez_t = persist.tile([P, NBT], F32)
        fpw_t = persist.tile([P, NBT, 5], F32)
        common_b = persist.tile([P, BH], MID)
        numer_b = persist.tile([P, BH], MID)
        wsum_b = persist.tile([P, BH], MID)
        common_h = persist.tile([P, 4, BC], MID)
        wsum_h = persist.tile([P, 4, BC], MID)
        enh_sb = persist.tile([P, 4, BC], MID)
        id_sb = persist.tile([P, P], FP16)
        ones_row = persist.tile([1, P], FP16)
        warmz = persist.tile([1, BC], MID)
        b1_sb = {}
        b2r_sb = {}
        wgp_sb = {}
        for name, _, K, dh in ENCS:
            b1_sb[name] = persist.tile([P, dh // P], F32, name=f"b1sb_{name}")
            b2r_sb[name] = persist.tile([1, H], FP16, name=f"b2r_{name}")
            wgp_sb[name] = persist.tile([P, dh // P, 5], FP16, name=f"wgp_{name}")
        z0_sb = persist.tile([5, 1], F32)
        enhb_sb = persist.tile([P, 4], F32)
        fusb_sb = persist.tile([P, 4], F32)
        ew_t = persist.tile([P, 4, 512], FP16, name="ew_t")
        fw_lo = persist.tile([P, 4, 512], FP16, name="fw_lo")
        fw_hi = persist.tile([P, 4, 512], FP16, name="fw_hi")

        nc.vector.memset(ones_row, 1.0)
        nc.vector.memset(warmz, 0.0)
        # PE warmup during the DMA preamble (K=1 matmuls, no DMA deps)
        for _wu in range(16):
            wu_ps = psum_l2.tile([P, H], F32, tag="l2ps", name=f"warm{_wu}")
            nc.tensor.matmul(wu_ps, ones_row, warmz, start=True, stop=True)
        nc.gpsimd.dma_start(id_sb, id128.ap())
        for name, _, K, dh in ENCS:
            nc.scalar.dma_start(b1_sb[name], b1[name].ap())
            nc.scalar.dma_start(b2r_sb[name], b2r[name].ap())
            nc.gpsimd.dma_start(
                wgp_sb[name], wgp[name].ap().rearrange("(ko p) m -> p ko m", p=P)
            )
        nc.gpsimd.dma_start(z0_sb, z0.ap())
        nc.scalar.dma_start(enhb_sb, enh_b.ap())
        nc.scalar.dma_start(fusb_sb, fus_b.ap())

        xt_view = xt.ap().rearrange("(ko p) n -> p ko n", p=P)

        def fps_blk(i, bt):  # [P, H] flat slice of encoder i, b-tile bt
            return fps_bt[:, i, bt * H : (bt + 1) * H]

        # ================= Phase A: encoders + pair prep ==================
        z_ps = psum_z.tile([5, BC], F32, tag="zps", name="zgate")
        Z_MM_TOTAL = sum(e[3] // P for e in ENCS)  # 15
        z_mm_done = 0

        numer_chains = [[] for _ in range(NBT)]
        n_enc_done = 0
        done_encs = []
        for name in ORDER:
            ei, (_, _, K, dh) = ENC_BY_NAME[name]
            M = dh // P
            # ---- layer 1 (h-major) ----
            psums = [
                psum_l1.tile([P, BC], F32, tag="mmps", name=f"l1_{name}_{m}")
                for m in range(M)
            ]
            h_sb = h_pool.tile([P, 4, BC], FP16, tag="htile")
            kdone = 0
            for kc0 in range(0, K, 4):
                kn = min(4, K - kc0)
                xt_t = xt_pool.tile([P, 4, BC], FP16, tag="xt")
                nc.sync.dma_start(
                    xt_t[:, :kn, :],
                    xt_view[:, XT_OFF[ei] + kc0 : XT_OFF[ei] + kc0 + kn, :],
                )
                w1_t = w_pool.tile([P, 4, 512], FP16, tag="w1")
                nc.sync.dma_start(
                    w1_t[:, :kn, :dh],
                    w1[name].ap()[kc0 * P : (kc0 + kn) * P, :].rearrange(
                        "(ko p) m -> p ko m", p=P
                    ),
                )
                for m in range(M):
                    for k in range(kn):
                        nc.tensor.matmul(
                            psums[m],
                            w1_t[:, k, m * P : (m + 1) * P],
                            xt_t[:, k, :],
                            start=(kdone + k == 0),
                            stop=(kdone + k == K - 1),
                        )
                kdone += kn
            for m in range(M):
                nc.scalar.activation(
                    h_sb[:, m, :], psums[m], AF.Relu, bias=b1_sb[name][:, m : m + 1]
                )
            # ---- gate partial ----
            for k in range(M):
                nc.tensor.matmul(
                    z_ps,
                    wgp_sb[name][:, k, :],
                    h_sb[:, k, :],
                    start=(z_mm_done == 0),
                    stop=(z_mm_done + 1 == Z_MM_TOTAL),
                )
                z_mm_done += 1
            # ---- layer 2 (b-major) ----
            w2_t = w_pool.tile([P, 4, 512], FP16, tag="w1")
            nc.sync.dma_start(
                w2_t[:, :M, :], w2[name].ap().rearrange("(ko p) m -> p ko m", p=P)
            )
            for bt in range(NBT):
                ps = psum_l2.tile([P, H], F32, tag="l2ps", name=f"l2_{name}_{bt}")
                for k in range(M):
                    nc.tensor.matmul(
                        ps,
                        h_sb[:, k, bt * P : (bt + 1) * P],
                        w2_t[:, k, :],
                        start=(k == 0),
                        stop=False,
                    )
                nc.tensor.matmul(
                    ps, ones_row[0:1, :], b2r_sb[name][0:1, :], start=False, stop=True
                )
                nc.scalar.activation(fps_blk(ei, bt), ps, AF.Copy)
                sq = sq_pool.tile([P, H], MID, tag="sq")
                nc.scalar.activation(
                    sq,
                    fps_blk(ei, bt),
                    AF.Square,
                    accum_out=stats[:, bt, 10 + ei : 11 + ei],
                )
            # ---- per-encoder ln(ss) column for the pair-sim chain ----
            nc.scalar.activation(
                lss_t[:, :, ei : ei + 1], stats[:, :, 10 + ei : 11 + ei], AF.Ln
            )
            # ---- pair pipeline: prod(+d), sum via DMA-accum, msum, e, numer
            is_last = name == ORDER[-1]
            for pn, prev in enumerate(done_encs):
                pkey = (min(prev, ei), max(prev, ei))
                pr = PAIR_IDX[pkey]
                i1, i2 = pkey
                prod = scr_pool.tile([P, BH], MID, tag="prod", name=f"prod{pr}")
                for bt in range(NBT):
                    nc.vector.scalar_tensor_tensor(
                        prod[:, bt * H : (bt + 1) * H],
                        in0=fps_blk(i1, bt),
                        scalar=0.0,
                        in1=fps_blk(i2, bt),
                        op0=ALU.add,
                        op1=ALU.mult,
                        accum_out=stats[:, bt, pr : pr + 1],
                    )
                sum_t = scr_pool.tile([P, BH], MID, tag="sum", name=f"sum{pr}")
                nc.gpsimd.tensor_add(sum_t, fps_bt[:, i1, :], fps_bt[:, i2, :])
                # e_p = [d>0]*exp(d/(||a||*||b||)) — tiny per-pair columns
                nc.vector.tensor_add(
                    pl_t[:, :, pr : pr + 1],
                    lss_t[:, :, i1 : i1 + 1],
                    lss_t[:, :, i2 : i2 + 1],
                )
                nc.scalar.activation(
                    invnn_t[:, :, pr : pr + 1], pl_t[:, :, pr : pr + 1],
                    AF.Exp, scale=-0.5,
                )
                nc.vector.tensor_mul(
                    sims_t[:, :, pr : pr + 1], stats[:, :, pr : pr + 1],
                    invnn_t[:, :, pr : pr + 1],
                )
                nc.scalar.activation(
                    es_t[:, :, pr : pr + 1], sims_t[:, :, pr : pr + 1], AF.Exp
                )
                nc.vector.scalar_tensor_tensor(
                    e_t[:, :, pr : pr + 1],
                    in0=stats[:, :, pr : pr + 1],
                    scalar=0.0,
                    in1=es_t[:, :, pr : pr + 1],
                    op0=ALU.is_gt,
                    op1=ALU.mult,
                )
                # msum + numer chain terms per b-tile (all DVE: [P,512]
                # DVE ops are reliably fast; gpsimd TS measured ~10us/op)
                eng = nc.vector
                for bt in range(NBT):
                    pslc = prod[:, bt * H : (bt + 1) * H]
                    sslc = sum_t[:, bt * H : (bt + 1) * H]
                    mslc = msum[:, pr, bt * H : (bt + 1) * H]
                    mask = msk_pool.tile([P, H], MID, tag="mask",
                                         name=f"mask{pr}_{bt}")
                    eng.tensor_scalar(
                        mask, in0=pslc, scalar1=0.0, scalar2=None, op0=ALU.is_gt
                    )
                    eng.tensor_mul(mslc, mask, sslc)
                    # numer += e_p * msum_p (affine_then_add chain, DVE)
                    nch = numer_chains[bt]
                    if len(nch) == 0:
                        seed = cb_pool.tile([P, H], MID, tag=f"nb{bt}",
                                            name=f"nseed{bt}")
                        nc.vector.tensor_scalar(
                            seed, in0=mslc,
                            scalar1=e_t[:, bt, pr : pr + 1], scalar2=None,
                            op0=ALU.mult,
                        )
                        nch.append(seed)
                    else:
                        lastn = len(nch)
                        dst = (
                            numer_b[:, bt * H : (bt + 1) * H]
                            if lastn == 9
                            else cb_pool.tile([P, H], MID, tag=f"nb{bt}",
                                              name=f"nb{bt}_{lastn}")
                        )
                        nc.vector.affine_then_add(
                            dst, mslc, nch[-1],
                            scale=e_t[:, bt, pr : pr + 1], bias=0.0,
                        )
                        nch.append(dst)
            # ---- running S on gpsimd ----
            cur = fps_bt[:, ei, :]
            if n_enc_done == 0:
                s_prev = cur
            elif n_enc_done == len(ORDER) - 1:
                nc.gpsimd.tensor_add(s_b, s_prev, cur)
            else:
                s_new = msk_pool.tile([P, BH], MID, tag="sacc",
                                      name=f"sacc{n_enc_done}")
                nc.gpsimd.tensor_add(s_new, s_prev, cur)
                s_prev = s_new
            done_encs.append(ei)
            n_enc_done += 1
            if n_enc_done == 2:
                nc.sync.dma_start(
                    ew_t, enh_w.ap().rearrange("(ko p) m -> p ko m", p=P)
                )
                fw_view = fus_w.ap().rearrange("(ko p) m -> p ko m", p=P)
                nc.sync.dma_start(fw_lo, fw_view[:, 0:4, :])
                nc.sync.dma_start(fw_hi, fw_view[:, 4:8, :])

        # ================= Phase B =================
        # fpw chain (needs z only)
        ez_h = cb_pool.tile([5, BC], MID, tag="ezh", name="ez_h")
        nc.scalar.activation(ez_h, z_ps, AF.Exp, bias=z0_sb[0:5, :])
        for bt in range(NBT):
            tps = psum_l1.tile([P, 8], FP16, tag="mmps", name=f"ezT{bt}")
            nc.tensor.transpose(
                tps[:, 0:5], ez_h[0:5, bt * P : (bt + 1) * P], id_sb[0:5, 0:5]
            )
            nc.scalar.activation(
                ez_bt[:, bt, :], tps[:, 0:5], AF.Copy,
                accum_out=sez_t[:, bt : bt + 1],
            )
        nc.vector.reciprocal(rsez_t, sez_t)

        # wsum chains: muls on ACT with UNNORMALIZED ez scale, adds on DVE,
        # then one TS by 1/sez
        for bt in range(NBT):
            wts = []
            for i in range(5):
                wt = wt_pool.tile([P, H], MID, tag="wt", name=f"wt{bt}_{i}")
                nc.scalar.activation(
                    wt, fps_blk(i, bt), AF.Copy, scale=ez_bt[:, bt, i : i + 1]
                )
                wts.append(wt)
            u1 = cb_pool.tile([P, H], MID, tag="wu", name=f"wu1_{bt}")
            u2 = cb_pool.tile([P, H], MID, tag="wu2", name=f"wu2_{bt}")
            nc.vector.tensor_add(u1, wts[0], wts[1])
            nc.vector.tensor_add(u2, wts[2], wts[3])
            u3 = cb_pool.tile([P, H], MID, tag="wu", name=f"wu3_{bt}")
            nc.vector.tensor_add(u3, u1, u2)
            u4 = cb_pool.tile([P, H], MID, tag="wu2", name=f"wu4_{bt}")
            nc.vector.tensor_add(u4, u3, wts[4])
            nc.vector.tensor_scalar(
                wsum_b[:, bt * H : (bt + 1) * H], in0=u4,
                scalar1=rsez_t[:, bt : bt + 1], scalar2=None, op0=ALU.mult,
            )
            # transpose wsum b-tile to h-major right away
            for ht in range(4):
                tps = psum_l1.tile([P, P], FP16, tag="mmps", name=f"wT{bt}_{ht}")
                nc.tensor.transpose(
                    tps, wsum_b[:, bt * H + ht * P : bt * H + (ht + 1) * P], id_sb
                )
                nc.scalar.activation(
                    wsum_h[:, ht, bt * P : (bt + 1) * P], tps, AF.Copy
                )

        # den / mf / common finishing (per-pair e columns already done)
        for bt in range(NBT):
            dj = sq_pool.tile([P, 16], MID, tag="dj", name=f"den{bt}")
            nc.vector.tensor_scalar(
                dj[:, 0:10], in0=e_t[:, bt, :], scalar1=0.0, scalar2=0.0,
                op0=ALU.add, op1=ALU.add, accum_out=den_t[:, bt : bt + 1],
            )
        nc.vector.tensor_scalar(
            mf_t, in0=den_t, scalar1=0.0, scalar2=0.2, op0=ALU.is_le, op1=ALU.mult
        )
        nc.vector.tensor_scalar_max(den1_t, den_t, 1.0)
        nc.vector.reciprocal(rden_t, den1_t)
        nc.vector.tensor_scalar_mul(r05_t, rden_t, 0.5)

        # ================= Phase C: finish common per b-tile ==============
        for bt in range(NBT):
            ctmp = cb_pool.tile([P, H], MID, tag="cb", name=f"ctmp{bt}")
            nc.vector.tensor_scalar(
                ctmp, in0=numer_b[:, bt * H : (bt + 1) * H],
                scalar1=r05_t[:, bt : bt + 1], scalar2=None, op0=ALU.mult,
            )
            nc.vector.affine_then_add(
                common_b[:, bt * H : (bt + 1) * H],
                s_b[:, bt * H : (bt + 1) * H], ctmp,
                scale=mf_t[:, bt : bt + 1], bias=0.0,
            )
            for ht in range(4):
                tps = psum_l1.tile([P, P], FP16, tag="mmps", name=f"cT{bt}_{ht}")
                nc.tensor.transpose(
                    tps, common_b[:, bt * H + ht * P : bt * H + (ht + 1) * P], id_sb
                )
                nc.scalar.activation(
                    common_h[:, ht, bt * P : (bt + 1) * P], tps, AF.Copy
                )

        # fus-lo accumulation (PE work spread through C); psums held open
        fus_ps = [
            psum_l2.tile([P, BC], F32, tag="l2ps", name=f"fus_{m}")
            for m in range(3)
        ]
        fus_ps.append(psum_z.tile([P, BC], F32, tag="zps", name="fus_3"))
        for m in range(4):
            for k in range(4):
                nc.tensor.matmul(
                    fus_ps[m],
                    fw_lo[:, k, m * P : (m + 1) * P],
                    wsum_h[:, k, :],
                    start=(k == 0),
                    stop=False,
                )

        # ================= Phase D: enhance + fuse (h-major) ===============
        for m in range(4):
            ps = psum_l1.tile([P, BC], F32, tag="mmps", name=f"enh_{m}")
            for k in range(4):
                nc.tensor.matmul(
                    ps,
                    ew_t[:, k, m * P : (m + 1) * P],
                    common_h[:, k, :],
                    start=(k == 0),
                    stop=(k == 3),
                )
            gate = gate_pool.tile([P, BC], MID, tag="gate")
            nc.scalar.activation(gate, ps, AF.Sigmoid, bias=enhb_sb[:, m : m + 1])
            nc.vector.tensor_mul(enh_sb[:, m, :], common_h[:, m, :], gate)

        out_view = out.ap().rearrange("(m p) n -> p m n", p=P)
        for m in range(4):
            for k in range(4):
                nc.tensor.matmul(
                    fus_ps[m],
                    fw_hi[:, k, m * P : (m + 1) * P],
                    enh_sb[:, k, :],
                    start=False,
                    stop=(k == 3),
                )
            o_sb = gate_pool.tile([P, BC], F32, tag="osb")
            nc.scalar.activation(
                o_sb, fus_ps[m], AF.Identity, bias=fusb_sb[:, m : m + 1]
            )
            nc.sync.dma_start(out_view[:, m, :], o_sb)


def prep_inputs(inputs):
    """Host-side: build the per-core in_maps from full inputs."""
    f16 = np.float16
    x = np.asarray(inputs["fp_features"], np.float32)

    def pad_rows(a, rows):
        a = np.asarray(a, np.float32)
        if a.shape[0] == rows:
            return a
        outp = np.zeros((rows, a.shape[1]), np.float32)
        outp[: a.shape[0]] = a
        return outp

    xt_full = np.zeros((XT_K * P, B), np.float32)
    offs_in = np.cumsum([0, AP_D, MA_D, MB_D, MC_D])
    for ei, (name, din, K, dh) in enumerate(ENCS):
        seg = x[:, offs_in[ei] : offs_in[ei] + din]
        xt_full[XT_OFF[ei] * P : XT_OFF[ei] * P + din, :] = np.ascontiguousarray(seg.T)
    xt_full = xt_full.astype(f16)

    wg_w = np.asarray(inputs["wg_w"], np.float32)
    wg_b = np.asarray(inputs["wg_b"], np.float32)
    common_map = {}
    z0v = wg_b.reshape(5).copy()
    for ei, (name, din, K, dh) in enumerate(ENCS):
        common_map[f"w1_{name}"] = pad_rows(inputs[f"{name}_w1"], K * P).astype(f16)
        w2f = np.asarray(inputs[f"{name}_w2"], np.float32)
        common_map[f"w2_{name}"] = w2f.astype(f16)
        common_map[f"b1_{name}"] = (
            np.asarray(inputs[f"{name}_b1"], np.float32).reshape(dh // P, P).T.copy()
        )
        b2f = np.asarray(inputs[f"{name}_b2"], np.float32)
        common_map[f"b2r_{name}"] = b2f.reshape(1, H).astype(f16)
        wg_i = wg_w[ei * H : (ei + 1) * H, :]
        common_map[f"wgp_{name}"] = (w2f @ wg_i).astype(f16)
        z0v += b2f @ wg_i
    common_map["z0"] = z0v.reshape(5, 1).astype(np.float32)
    common_map["id128"] = np.eye(P, dtype=f16)
    common_map["enh_w"] = np.asarray(inputs["enh_w"], f16)
    common_map["enh_b"] = np.asarray(inputs["enh_b"], np.float32).reshape(4, P).T.copy()
    common_map["fus_w"] = np.asarray(inputs["fus_w"], f16)
    common_map["fus_b"] = np.asarray(inputs["fus_b"], np.float32).reshape(4, P).T.copy()

    in_maps = []
    for c in range(N_CORES):
        m = dict(common_map)
        m["xt"] = np.ascontiguousarray(xt_full[:, c * BC : (c + 1) * BC])
        in_maps.append(m)
    return in_maps


_NC_CACHE = None


def kernel(**inputs) -> np.ndarray:
    global _NC_CACHE
    if _NC_CACHE is None:
        _NC_CACHE = build_bass()
    nc = _NC_CACHE
    in_maps = prep_inputs(inputs)
    res = run_bass_kernel_spmd(nc, in_maps, core_ids=list(range(N_CORES)))
    outs = [res.results[c]["out"] for c in range(N_CORES)]
    full = np.concatenate([o.T for o in outs], axis=0)
    return np.ascontiguousarray(full.astype(np.float32))
